# revision 6
# baseline (speedup 1.0000x reference)
"""Banded-causal complex attention on 8 Trainium2 NeuronCores.

Strategy: data-parallel over batch (B=8 -> 1 batch per core). Per core:
  - host feeds x[b].T ([D,S]) so projections run as W.T @ x.T with weights
    stationary on the PE.
  - Q is packed [Wqr|Wqi]*scale^2*temp, K is packed [Wkr|-Wki]: the complex
    score real part (qr.kr - qi.ki)*scale*temp becomes ONE K=128 matmul.
  - scores are computed transposed (sT[key, query]) per 128x128 block pair;
    the band+causal mask reduces to two triangular affine_selects.
  - softmax skips the max-subtraction (scores are O(5); masked entries are
    exactly zero after the select) and gets row-sums for free by appending a
    ones column to V in the P@V matmul.
  - only 2 key-blocks per query-block are touched (129-wide causal band),
    i.e. 1/8 of the dense score work.
"""

import numpy as np

B, S, D, KD = 8, 2048, 512, 64
P = 128              # partition size / query block
NB = S // P          # 16 query/key blocks
DCH = D // P         # 4 contraction chunks
NCORES = 8

_CACHE = {}
TRACE_KWARGS = {}    # test harness may set e.g. {"trace": True, "tmpdir": ...}


def _build_nc():
    import concourse.bacc as bacc
    import concourse.tile as tile
    import concourse.mybir as mybir
    from concourse.bass import ts

    f32 = mybir.dt.float32
    nc = bacc.Bacc(None)

    xT = nc.declare_dram_parameter("xT", [D, S], f32, isOutput=False)
    wq = nc.declare_dram_parameter("wq", [D, P], f32, isOutput=False)
    wk = nc.declare_dram_parameter("wk", [D, P], f32, isOutput=False)
    wv = nc.declare_dram_parameter("wv", [D, KD], f32, isOutput=False)
    pq = nc.declare_dram_parameter("pq", [P, S], f32, isOutput=False)
    pk = nc.declare_dram_parameter("pk", [P, S], f32, isOutput=False)
    bv = nc.declare_dram_parameter("bv", [KD, 1], f32, isOutput=False)
    out = nc.declare_dram_parameter("out", [S, KD], f32, isOutput=True)

    ident = nc.inline_tensor(np.eye(KD, dtype=np.float32), name="ident64")

    with tile.TileContext(nc) as tc:
        with (
            tc.tile_pool(name="consts", bufs=1) as consts,
            tc.tile_pool(name="persist", bufs=1) as persist,
            tc.tile_pool(name="work", bufs=3) as work,
        ):
            ident_sb = consts.tile([KD, KD], f32)
            nc.sync.dma_start(out=ident_sb, in_=ident[:])
            bv_sb = consts.tile([KD, 1], f32)
            nc.sync.dma_start(out=bv_sb, in_=bv[:])

            wq_sb = consts.tile([P, DCH, P], f32)
            nc.sync.dma_start(out=wq_sb, in_=wq.rearrange("(c p) m -> p c m", p=P))
            wk_sb = consts.tile([P, DCH, P], f32)
            nc.sync.dma_start(out=wk_sb, in_=wk.rearrange("(c p) m -> p c m", p=P))
            wv_sb = consts.tile([P, DCH, KD], f32)
            nc.sync.dma_start(out=wv_sb, in_=wv.rearrange("(c p) m -> p c m", p=P))

            pq_sb = persist.tile([P, S], f32)
            pk_sb = persist.tile([P, S], f32)
            nc.sync.dma_start(out=pq_sb, in_=pq[:])
            nc.sync.dma_start(out=pk_sb, in_=pk[:])

            # x.T in SBUF, loaded in halves so compute can start early and
            # DMA overlaps the projection matmuls.
            xT_sb = persist.tile([P, DCH, S], f32)
            for half in range(2):
                cols = slice(half * (S // 2), (half + 1) * (S // 2))
                for c in range(DCH):
                    nc.sync.dma_start(
                        out=xT_sb[:, c, cols],
                        in_=xT[c * P : (c + 1) * P, cols],
                    )

            qT_sb = persist.tile([P, S], f32)
            kT_sb = persist.tile([P, S], f32)
            vT_sb = persist.tile([KD, S], f32)

            # ---- projections: qT/kT = W.T @ xT + pos, vT = Wv.T @ xT + bv
            NSL = 512  # psum free-dim per matmul (one fp32 bank)
            with tc.tile_pool(name="ps_proj", bufs=6, space="PSUM") as ps_proj:
                for half in range(2):
                    for grp in range(3):  # 0=q, 1=k, 2=v
                        w_g = (wq_sb, wk_sb, wv_sb)[grp]
                        m = P if grp < 2 else KD
                        pss = []
                        for n2 in range(2):
                            pss.append(
                                ps_proj.tile([m, NSL], f32, tag="ps", name="ps")
                            )
                        for c in range(DCH):
                            for n2 in range(2):
                                col0 = half * 1024 + n2 * NSL
                                nc.tensor.matmul(
                                    pss[n2],
                                    w_g[:, c, :m],
                                    xT_sb[:, c, col0 : col0 + NSL],
                                    start=(c == 0),
                                    stop=(c == DCH - 1),
                                )
                        for n2 in range(2):
                            col0 = half * 1024 + n2 * NSL
                            sl = slice(col0, col0 + NSL)
                            if grp == 0:
                                nc.vector.tensor_add(qT_sb[:, sl], pss[n2], pq_sb[:, sl])
                            elif grp == 1:
                                nc.vector.tensor_add(kT_sb[:, sl], pss[n2], pk_sb[:, sl])
                            else:
                                nc.scalar.activation(
                                    out=vT_sb[:, sl],
                                    in_=pss[n2],
                                    func=mybir.ActivationFunctionType.Identity,
                                    bias=bv_sb,
                                    scale=1.0,
                                )

            # ---- v_aug[key, block, 0:64] = v natural; col 64 = 1.0 (rowsum col)
            v_aug = persist.tile([P, NB, KD + 1], f32)
            nc.vector.memset(v_aug[:, :, KD : KD + 1], 1.0)

            with (
                tc.tile_pool(name="ps_tp", bufs=2, space="PSUM") as ps_tp,
                tc.tile_pool(name="ps_s", bufs=3, space="PSUM") as ps_s,
                tc.tile_pool(name="ps_o", bufs=2, space="PSUM") as ps_o,
            ):
                for t in range(NB):
                    tp = ps_tp.tile([P, KD], f32)
                    nc.tensor.transpose(tp, vT_sb[:, ts(t, P)], ident_sb)
                    nc.vector.tensor_copy(v_aug[:, t, 0:KD], tp)

                # ---- attention: sT[key c, query r] blocks, masked exp, P@V
                s_tiles = {}

                def attend(qb):
                    # query block qb attends key blocks qb-1 (h=0) and qb (h=1)
                    s_ps = s_tiles.pop(qb)
                    p_sb = work.tile([P, 2, P], f32, tag="p_sb")
                    if qb > 0:
                        nc.scalar.activation(
                            out=p_sb, in_=s_ps, func=mybir.ActivationFunctionType.Exp
                        )
                        # off-diag block: keep keys c >= r (band lower edge)
                        nc.gpsimd.affine_select(
                            out=p_sb[:, 0, :], in_=p_sb[:, 0, :],
                            pattern=[[-1, P]], compare_op=mybir.AluOpType.is_ge,
                            fill=0.0, base=0, channel_multiplier=1,
                        )
                    else:
                        nc.scalar.activation(
                            out=p_sb[:, 1, :], in_=s_ps[:, 1, :],
                            func=mybir.ActivationFunctionType.Exp,
                        )
                    # diagonal block: keep keys c <= r (causal), i.e. r - c >= 0
                    nc.gpsimd.affine_select(
                        out=p_sb[:, 1, :], in_=p_sb[:, 1, :],
                        pattern=[[1, P]], compare_op=mybir.AluOpType.is_ge,
                        fill=0.0, base=0, channel_multiplier=-1,
                    )
                    o_ps = ps_o.tile([P, KD + 1], f32)
                    halves = [(1, qb)] if qb == 0 else [(0, qb - 1), (1, qb)]
                    for i, (h, kb2) in enumerate(halves):
                        nc.tensor.matmul(
                            o_ps,
                            p_sb[:, h, :],
                            v_aug[:, kb2, :],
                            start=(i == 0),
                            stop=(i == len(halves) - 1),
                        )
                    r_sb = work.tile([P, 1], f32, tag="r_sb")
                    nc.vector.reciprocal(r_sb, o_ps[:, KD : KD + 1])
                    out_sb = work.tile([P, KD], f32, tag="out_sb")
                    nc.vector.tensor_scalar_mul(out_sb, o_ps[:, 0:KD], r_sb)
                    nc.sync.dma_start(out=out[ts(qb, P), :], in_=out_sb)

                for kb in range(NB):
                    # one kT weight-load serves both dependent query blocks
                    if kb == 0:
                        s_tiles[0] = ps_s.tile([P, 2, P], f32, tag="s_ps", name="s_ps")
                    if kb + 1 < NB:
                        s_tiles[kb + 1] = ps_s.tile([P, 2, P], f32, tag="s_ps", name="s_ps")
                        nc.tensor.matmul(
                            s_tiles[kb + 1][:, 0, :],
                            kT_sb[:, ts(kb, P)],
                            qT_sb[:, ts(kb + 1, P)],
                            start=True, stop=True,
                        )
                    nc.tensor.matmul(
                        s_tiles[kb][:, 1, :],
                        kT_sb[:, ts(kb, P)],
                        qT_sb[:, ts(kb, P)],
                        start=True, stop=True,
                    )
                    attend(kb)

    nc.finalize()
    return nc


def _prep_core_inputs(inputs):
    g = lambda k: np.asarray(inputs[k], dtype=np.float32)
    x = g("x")
    scale = 1.0 / np.sqrt(np.float32(KD))
    temp = float(np.asarray(inputs["temperature"]).reshape(-1)[0])
    alpha = scale * temp  # folded (softmax temp) * (score scale)

    wq = np.concatenate([g("Wqr"), g("Wqi")], axis=1) * (scale * alpha)
    pq = np.concatenate(
        [
            g("pos_qr") * alpha + g("bqr") * (scale * alpha),
            g("pos_qi") * alpha + g("bqi") * (scale * alpha),
        ],
        axis=1,
    ).T  # [128, S]
    wk = np.concatenate([g("Wkr"), -g("Wki")], axis=1)
    pk = np.concatenate(
        [g("pos_kr") + g("bkr"), -(g("pos_ki") + g("bki"))], axis=1
    ).T
    wv = g("Wv")
    bv = g("bv").reshape(KD, 1)

    shared = {
        "wq": np.ascontiguousarray(wq, dtype=np.float32),
        "wk": np.ascontiguousarray(wk, dtype=np.float32),
        "wv": np.ascontiguousarray(wv, dtype=np.float32),
        "pq": np.ascontiguousarray(pq, dtype=np.float32),
        "pk": np.ascontiguousarray(pk, dtype=np.float32),
        "bv": np.ascontiguousarray(bv, dtype=np.float32),
    }
    in_maps = []
    for b in range(NCORES):
        m = dict(shared)
        m["xT"] = np.ascontiguousarray(x[b].T, dtype=np.float32)
        in_maps.append(m)
    return in_maps


def kernel(**inputs):
    from concourse.bass_utils import run_bass_kernel_spmd

    nc = _CACHE.get("nc")
    if nc is None:
        nc = _CACHE["nc"] = _build_nc()
    in_maps = _prep_core_inputs(inputs)
    res = run_bass_kernel_spmd(
        nc, in_maps, core_ids=list(range(NCORES)), **TRACE_KWARGS
    )
    _CACHE["last_result"] = res
    return np.stack([res.results[b]["out"] for b in range(NCORES)], axis=0)


# revision 11
# speedup vs baseline: 1.4770x; 1.4770x over previous
"""Banded-causal complex attention on 8 Trainium2 NeuronCores.

Strategy: data-parallel over batch (B=8 -> 1 batch per core). Per core:
  - host feeds x[b].T ([D,S]) so projections run as W.T @ x.T with weights
    stationary on the PE.
  - Q is packed [Wqr|Wqi]*scale^2*temp, K is packed [Wkr|-Wki]: the complex
    score real part (qr.kr - qi.ki)*scale*temp becomes ONE K=128 matmul.
  - all matmuls run in float32r (single-pass fp32 PE mode, 4x the fp32 rate
    at moving-dim >= 256); scores are computed transposed (sT[key, query])
    as one N=256 matmul per key block covering both dependent query blocks.
  - band+causal masking is two triangular affine_selects on GpSimd.
  - softmax skips the max-subtraction (scores are O(5); masked entries are
    exactly zero after the select) and gets row-sums for free by appending a
    ones column to V in the P@V matmul.
  - only 2 key-blocks per query-block are touched (129-wide causal band),
    i.e. 1/8 of the dense score work.
"""

import numpy as np

B, S, D, KD = 8, 2048, 512, 64
P = 128              # partition size / query block
NB = S // P          # 16 query/key blocks
DCH = D // P         # 4 contraction chunks
NCORES = 8

_CACHE = {}
TRACE_KWARGS = {}    # test harness may set e.g. {"trace": True, "tmpdir": ...}


def _build_nc():
    import concourse.bacc as bacc
    import concourse.tile as tile
    import concourse.mybir as mybir
    from concourse.bass import ts

    f32 = mybir.dt.float32
    f32r = mybir.dt.float32r
    r = lambda ap: ap.bitcast(f32r)
    nc = bacc.Bacc(None)

    xT = nc.declare_dram_parameter("xT", [D, S], f32r, isOutput=False)
    wq = nc.declare_dram_parameter("wq", [D, P], f32r, isOutput=False)
    wk = nc.declare_dram_parameter("wk", [D, P], f32r, isOutput=False)
    wv = nc.declare_dram_parameter("wv", [D, KD], f32r, isOutput=False)
    pq = nc.declare_dram_parameter("pq", [P, S], f32, isOutput=False)
    pk = nc.declare_dram_parameter("pk", [P, S], f32, isOutput=False)
    bv = nc.declare_dram_parameter("bv", [KD, 1], f32, isOutput=False)
    out = nc.declare_dram_parameter("out", [S, KD], f32, isOutput=True)

    ident = nc.inline_tensor(np.eye(KD, dtype=np.float32), name="ident64")

    with tile.TileContext(nc) as tc:
        with (
            tc.tile_pool(name="consts", bufs=1) as consts,
            tc.tile_pool(name="persist", bufs=1) as persist,
            tc.tile_pool(name="work", bufs=4) as work,
        ):
            # small/constant loads on gpsimd's queues; bulk xT on sync's
            wq_sb = consts.tile([P, DCH, P], f32r)
            nc.gpsimd.dma_start(out=wq_sb, in_=wq.rearrange("(c p) m -> p c m", p=P))
            wk_sb = consts.tile([P, DCH, P], f32r)
            nc.gpsimd.dma_start(out=wk_sb, in_=wk.rearrange("(c p) m -> p c m", p=P))
            wv_sb = consts.tile([P, DCH, KD], f32r)
            nc.gpsimd.dma_start(out=wv_sb, in_=wv.rearrange("(c p) m -> p c m", p=P))
            ident_sb = consts.tile([KD, KD], f32)
            nc.gpsimd.dma_start(out=ident_sb, in_=ident[:])
            bv_sb = consts.tile([KD, 1], f32)
            nc.gpsimd.dma_start(out=bv_sb, in_=bv[:])

            # x.T in SBUF, half-columns at a time so compute starts early
            xT_sb = persist.tile([P, DCH, S], f32r)
            for half in range(2):
                cols = slice(half * (S // 2), (half + 1) * (S // 2))
                for c in range(DCH):
                    nc.sync.dma_start(
                        out=xT_sb[:, c, cols],
                        in_=xT[c * P : (c + 1) * P, cols],
                    )

            pq_sb = persist.tile([P, S], f32)
            pk_sb = persist.tile([P, S], f32)
            nc.gpsimd.dma_start(out=pq_sb, in_=pq[:])
            nc.gpsimd.dma_start(out=pk_sb, in_=pk[:])

            qT_sb = persist.tile([P, S], f32r)
            kT_sb = persist.tile([P, S], f32r)
            vT_sb = persist.tile([KD, S], f32)

            # ---- projections: qT/kT = W.T @ xT + pos, vT = Wv.T @ xT + bv
            NSL = 512  # psum free-dim per matmul (one fp32 bank)
            with tc.tile_pool(name="ps_proj", bufs=6, space="PSUM") as ps_proj:
                for half in range(2):
                    for grp in range(3):  # 0=q, 1=k, 2=v
                        w_g = (wq_sb, wk_sb, wv_sb)[grp]
                        m = P if grp < 2 else KD
                        pss = []
                        for n2 in range(2):
                            pss.append(
                                ps_proj.tile([m, NSL], f32, tag="ps", name="ps")
                            )
                        for c in range(DCH):
                            for n2 in range(2):
                                col0 = half * 1024 + n2 * NSL
                                nc.tensor.matmul(
                                    pss[n2],
                                    w_g[:, c, :m],
                                    xT_sb[:, c, col0 : col0 + NSL],
                                    start=(c == 0),
                                    stop=(c == DCH - 1),
                                )
                        for n2 in range(2):
                            col0 = half * 1024 + n2 * NSL
                            sl = slice(col0, col0 + NSL)
                            if grp == 0:
                                nc.vector.tensor_add(qT_sb[:, sl], pss[n2], pq_sb[:, sl])
                            elif grp == 1:
                                nc.vector.tensor_add(kT_sb[:, sl], pss[n2], pk_sb[:, sl])
                            else:
                                nc.vector.tensor_scalar_add(
                                    vT_sb[:, sl], pss[n2], bv_sb
                                )

            # ---- v_aug[key, block, 0:64] = v natural; col 64 = 1.0 (rowsum col)
            v_aug = persist.tile([P, NB, KD + 2], f32r)
            ones_sb = consts.tile([P, 1], f32)
            nc.vector.memset(ones_sb, 1.0)
            nc.vector.tensor_copy(
                v_aug[:, :, KD : KD + 2], ones_sb.to_broadcast((P, NB, 2))
            )

            with (
                tc.tile_pool(name="ps_tp", bufs=2, space="PSUM") as ps_tp,
                tc.tile_pool(name="ps_s", bufs=3, space="PSUM") as ps_s,
                tc.tile_pool(name="ps_o", bufs=3, space="PSUM") as ps_o,
            ):
                for t in range(NB):
                    tp = ps_tp.tile([P, KD], f32)
                    nc.tensor.transpose(tp, vT_sb[:, ts(t, P)], ident_sb)
                    nc.vector.tensor_copy(v_aug[:, t, 0:KD], tp)

                # ---- attention: sT_kb[key c, query r] = scores.T for the two
                # query blocks (kb, kb+1) that attend key block kb; N=256 keeps
                # float32r on the fast path.
                s_tiles = {}

                def attend(qb):
                    # p halves: h=0 keys from block qb-1, h=1 keys from block qb
                    p_sb = work.tile([P, 2, P], f32r, tag="p_sb")
                    if qb > 0:
                        nc.scalar.activation(
                            out=p_sb[:, 0, :], in_=s_tiles[qb - 1][:, P : 2 * P],
                            func=mybir.ActivationFunctionType.Exp,
                        )
                        # off-diag block: keep keys c >= r (band lower edge)
                        nc.gpsimd.affine_select(
                            out=p_sb[:, 0, :], in_=p_sb[:, 0, :],
                            pattern=[[-1, P]], compare_op=mybir.AluOpType.is_ge,
                            fill=0.0, base=0, channel_multiplier=1,
                        )
                    nc.scalar.activation(
                        out=p_sb[:, 1, :], in_=s_tiles[qb][:, 0:P],
                        func=mybir.ActivationFunctionType.Exp,
                    )
                    # diagonal block: keep keys c <= r (causal), i.e. r - c >= 0
                    nc.gpsimd.affine_select(
                        out=p_sb[:, 1, :], in_=p_sb[:, 1, :],
                        pattern=[[1, P]], compare_op=mybir.AluOpType.is_ge,
                        fill=0.0, base=0, channel_multiplier=-1,
                    )
                    if qb > 1:
                        s_tiles.pop(qb - 2, None)
                    o_ps = ps_o.tile([P, KD + 2], f32)
                    halves = [(1, qb)] if qb == 0 else [(0, qb - 1), (1, qb)]
                    for i, (h, kb2) in enumerate(halves):
                        nc.tensor.matmul(
                            o_ps,
                            p_sb[:, h, :],
                            v_aug[:, kb2, :],
                            start=(i == 0),
                            stop=(i == len(halves) - 1),
                        )
                    r_sb = work.tile([P, 1], f32, tag="r_sb")
                    nc.vector.reciprocal(r_sb, o_ps[:, KD : KD + 1])
                    out_sb = work.tile([P, KD], f32, tag="out_sb")
                    nc.vector.tensor_scalar_mul(out_sb, o_ps[:, 0:KD], r_sb)
                    nc.sync.dma_start(out=out[ts(qb, P), :], in_=out_sb)

                for kb in range(NB):
                    # one matmul scores key block kb against query blocks kb,kb+1
                    ncols = 2 * P if kb + 1 < NB else P
                    s_tiles[kb] = ps_s.tile([P, 2 * P], f32, tag="s_ps", name="s_ps")
                    nc.tensor.matmul(
                        s_tiles[kb][:, 0:ncols],
                        kT_sb[:, ts(kb, P)],
                        qT_sb[:, kb * P : kb * P + ncols],
                        start=True, stop=True,
                    )
                    attend(kb)

    nc.finalize()
    return nc


def _prep_core_inputs(inputs):
    g = lambda k: np.asarray(inputs[k], dtype=np.float32)
    x = g("x")
    scale = 1.0 / np.sqrt(np.float32(KD))
    temp = float(np.asarray(inputs["temperature"]).reshape(-1)[0])
    alpha = scale * temp  # folded (softmax temp) * (score scale)

    wq = np.concatenate([g("Wqr"), g("Wqi")], axis=1) * (scale * alpha)
    pq = np.concatenate(
        [
            g("pos_qr") * alpha + g("bqr") * (scale * alpha),
            g("pos_qi") * alpha + g("bqi") * (scale * alpha),
        ],
        axis=1,
    ).T  # [128, S]
    wk = np.concatenate([g("Wkr"), -g("Wki")], axis=1)
    pk = np.concatenate(
        [g("pos_kr") + g("bkr"), -(g("pos_ki") + g("bki"))], axis=1
    ).T
    wv = g("Wv")
    bv = g("bv").reshape(KD, 1)

    shared = {
        "wq": np.ascontiguousarray(wq, dtype=np.float32),
        "wk": np.ascontiguousarray(wk, dtype=np.float32),
        "wv": np.ascontiguousarray(wv, dtype=np.float32),
        "pq": np.ascontiguousarray(pq, dtype=np.float32),
        "pk": np.ascontiguousarray(pk, dtype=np.float32),
        "bv": np.ascontiguousarray(bv, dtype=np.float32),
    }
    in_maps = []
    for b in range(NCORES):
        m = dict(shared)
        m["xT"] = np.ascontiguousarray(x[b].T, dtype=np.float32)
        in_maps.append(m)
    return in_maps


def kernel(**inputs):
    from concourse.bass_utils import run_bass_kernel_spmd

    nc = _CACHE.get("nc")
    if nc is None:
        nc = _CACHE["nc"] = _build_nc()
    in_maps = _prep_core_inputs(inputs)
    res = run_bass_kernel_spmd(
        nc, in_maps, core_ids=list(range(NCORES)), **TRACE_KWARGS
    )
    _CACHE["last_result"] = res
    return np.stack([res.results[b]["out"] for b in range(NCORES)], axis=0)


# revision 13
# speedup vs baseline: 1.7656x; 1.1954x over previous
"""Banded-causal complex attention on 8 Trainium2 NeuronCores.

Strategy: data-parallel over batch (B=8 -> 1 batch per core). Per core:
  - host feeds x[b].T ([D,S]) so projections run as W.T @ x.T with weights
    stationary on the PE.
  - Q is packed [Wqr|Wqi]*scale^2*temp, K is packed [Wkr|-Wki]: the complex
    score real part (qr.kr - qi.ki)*scale*temp becomes ONE K=128 matmul.
  - matmuls run in float32r (single-pass fp32 PE mode; needs even free dims
    and moving-dim >= 256 for the fast path).
  - scores are computed transposed: sT_kb[key c, query r] covers the two
    query blocks (kb, kb+1) that attend key block kb, one N=256 matmul each.
  - band+causal masking is two triangular affine_selects on GpSimd over the
    halves of exp(sT).
  - softmax skips the max-subtraction (scores are O(5); masked entries are
    exactly zero) and row-sums ride along as a ones column appended to V.
  - phases interleave: [proj half0 | attn kb 0-6 | proj half1 | attn kb 7-15]
    so the PE stays busy while the second half of x.T streams in.
"""

import numpy as np

B, S, D, KD = 8, 2048, 512, 64
P = 128              # partition size / query block
NB = S // P          # 16 query/key blocks
DCH = D // P         # 4 contraction chunks
NCORES = 8

_CACHE = {}
TRACE_KWARGS = {}    # test harness may set e.g. {"trace": True, "tmpdir": ...}


def _build_nc():
    import concourse.bacc as bacc
    import concourse.tile as tile
    import concourse.mybir as mybir
    from concourse.bass import ts

    f32 = mybir.dt.float32
    f32r = mybir.dt.float32r
    nc = bacc.Bacc(None)

    xT = nc.declare_dram_parameter("xT", [D, S], f32r, isOutput=False)
    wq = nc.declare_dram_parameter("wq", [D, P], f32r, isOutput=False)
    wk = nc.declare_dram_parameter("wk", [D, P], f32r, isOutput=False)
    wv = nc.declare_dram_parameter("wv", [D, KD], f32r, isOutput=False)
    pq = nc.declare_dram_parameter("pq", [P, S], f32, isOutput=False)
    pk = nc.declare_dram_parameter("pk", [P, S], f32, isOutput=False)
    bv = nc.declare_dram_parameter("bv", [KD, 1], f32, isOutput=False)
    out = nc.declare_dram_parameter("out", [S, KD], f32, isOutput=True)

    ident = nc.inline_tensor(np.eye(KD, dtype=np.float32), name="ident64")
    HS = S // 2

    with tile.TileContext(nc) as tc:
        with (
            tc.tile_pool(name="consts", bufs=1) as consts,
            tc.tile_pool(name="persist", bufs=1) as persist,
            tc.tile_pool(name="work", bufs=4) as work,
            tc.tile_pool(name="ps_proj", bufs=2, space="PSUM") as ps_proj,
            tc.tile_pool(name="ps_s", bufs=3, space="PSUM") as ps_s,
            tc.tile_pool(name="ps_small", bufs=3, space="PSUM") as ps_small,
        ):
            # warm the ACT exp table before it's on the critical path
            dummy = consts.tile([P, 2], f32)
            nc.vector.memset(dummy, 0.0)
            nc.scalar.activation(
                out=dummy, in_=dummy, func=mybir.ActivationFunctionType.Exp
            )

            # x.T: half-columns, 4 chunk-pieces each; issue from two DGE
            # engines so descriptor generation isn't serialized on one engine
            xT_sb = persist.tile([P, DCH, S], f32r)
            for c in range(DCH):
                nc.sync.dma_start(
                    out=xT_sb[:, c, 0:HS], in_=xT[c * P : (c + 1) * P, 0:HS]
                )
            for c in range(DCH):
                nc.scalar.dma_start(
                    out=xT_sb[:, c, HS:S], in_=xT[c * P : (c + 1) * P, HS:S]
                )

            # weights/pos/consts on gpsimd's queues
            wq_sb = consts.tile([P, DCH, P], f32r)
            nc.gpsimd.dma_start(out=wq_sb, in_=wq.rearrange("(c p) m -> p c m", p=P))
            wk_sb = consts.tile([P, DCH, P], f32r)
            nc.gpsimd.dma_start(out=wk_sb, in_=wk.rearrange("(c p) m -> p c m", p=P))
            wv_sb = consts.tile([P, DCH, KD], f32r)
            nc.gpsimd.dma_start(out=wv_sb, in_=wv.rearrange("(c p) m -> p c m", p=P))
            pq_sb = persist.tile([P, S], f32)
            pk_sb = persist.tile([P, S], f32)
            nc.gpsimd.dma_start(out=pq_sb[:, 0:HS], in_=pq[:, 0:HS])
            nc.gpsimd.dma_start(out=pk_sb[:, 0:HS], in_=pk[:, 0:HS])
            ident_sb = consts.tile([KD, KD], f32)
            nc.gpsimd.dma_start(out=ident_sb, in_=ident[:])
            bv_sb = consts.tile([KD, 1], f32)
            nc.gpsimd.dma_start(out=bv_sb, in_=bv[:])
            nc.gpsimd.dma_start(out=pq_sb[:, HS:S], in_=pq[:, HS:S])
            nc.gpsimd.dma_start(out=pk_sb[:, HS:S], in_=pk[:, HS:S])

            # qT padded by one block so every sT matmul is a uniform N=256
            qT_sb = persist.tile([P, S + P], f32r)
            kT_sb = persist.tile([P, S], f32r)
            vT_sb = persist.tile([KD, S], f32)
            zeros_sb = consts.tile([P, 1], f32)
            nc.vector.memset(zeros_sb, 0.0)
            nc.vector.tensor_copy(
                qT_sb[:, S : S + P], zeros_sb.to_broadcast((P, P))
            )

            # v_aug[key, block, 0:64] = v; col 64 = 1.0 (rowsum); col 65 pad
            v_aug = persist.tile([P, NB, KD + 2], f32r)
            ones_sb = consts.tile([P, 1], f32)
            nc.vector.memset(ones_sb, 1.0)
            nc.vector.tensor_copy(
                v_aug[:, :, KD : KD + 2], ones_sb.to_broadcast((P, NB, 2))
            )

            # per-query-block normalized outputs, DMA'd out 4 blocks at a time
            out_all = persist.tile([P, NB, KD], f32)
            out_r = out.rearrange("(q r) k -> r q k", r=P)

            NSL = 512

            def proj_half(half):
                for grp in range(3):  # 0=q, 1=k, 2=v
                    w_g = (wq_sb, wk_sb, wv_sb)[grp]
                    m = P if grp < 2 else KD
                    pss = [
                        ps_proj.tile([m, NSL], f32, tag="ps", name="ps")
                        for _ in range(2)
                    ]
                    for c in range(DCH):
                        for n2 in range(2):
                            col0 = half * HS + n2 * NSL
                            nc.tensor.matmul(
                                pss[n2],
                                w_g[:, c, :m],
                                xT_sb[:, c, col0 : col0 + NSL],
                                start=(c == 0),
                                stop=(c == DCH - 1),
                            )
                    for n2 in range(2):
                        col0 = half * HS + n2 * NSL
                        sl = slice(col0, col0 + NSL)
                        if grp == 0:
                            nc.vector.tensor_add(qT_sb[:, sl], pss[n2], pq_sb[:, sl])
                        elif grp == 1:
                            nc.vector.tensor_add(kT_sb[:, sl], pss[n2], pk_sb[:, sl])
                        else:
                            nc.vector.tensor_scalar_add(vT_sb[:, sl], pss[n2], bv_sb)

            def transpose_v(t):
                tp = ps_small.tile([P, KD], f32, tag="small", name="tp")
                nc.tensor.transpose(tp, vT_sb[:, ts(t, P)], ident_sb)
                nc.vector.tensor_copy(v_aug[:, t, 0:KD], tp)

            p_tiles = {}

            def score_block(kb):
                # sT_kb[c, r]: keys of block kb vs queries of blocks kb,kb+1
                s_ps = ps_s.tile([P, 2 * P], f32, tag="s", name="s_ps")
                nc.tensor.matmul(
                    s_ps,
                    kT_sb[:, ts(kb, P)],
                    qT_sb[:, kb * P : kb * P + 2 * P],
                    start=True, stop=True,
                )
                p_sb = work.tile([P, 2, P], f32r, tag="p_sb")
                nc.scalar.activation(
                    out=p_sb, in_=s_ps.rearrange("c (h r) -> c h r", h=2),
                    func=mybir.ActivationFunctionType.Exp,
                )
                # half 0 = diag block of qb=kb: keep keys c <= r
                nc.gpsimd.affine_select(
                    out=p_sb[:, 0, :], in_=p_sb[:, 0, :],
                    pattern=[[1, P]], compare_op=mybir.AluOpType.is_ge,
                    fill=0.0, base=0, channel_multiplier=-1,
                )
                # half 1 = off-diag block of qb=kb+1: keep keys c >= r
                nc.gpsimd.affine_select(
                    out=p_sb[:, 1, :], in_=p_sb[:, 1, :],
                    pattern=[[-1, P]], compare_op=mybir.AluOpType.is_ge,
                    fill=0.0, base=0, channel_multiplier=1,
                )
                p_tiles[kb] = p_sb

            def attend(qb):
                o_ps = ps_small.tile([P, KD + 2], f32, tag="small", name="o_ps")
                halves = [(p_tiles[qb], 0, qb)]
                if qb > 0:
                    halves.insert(0, (p_tiles[qb - 1], 1, qb - 1))
                for i, (pt, h, kb2) in enumerate(halves):
                    nc.tensor.matmul(
                        o_ps,
                        pt[:, h, :],
                        v_aug[:, kb2, :],
                        start=(i == 0),
                        stop=(i == len(halves) - 1),
                    )
                if qb > 1:
                    p_tiles.pop(qb - 2, None)
                r_sb = work.tile([P, 1], f32, tag="r_sb")
                nc.vector.reciprocal(r_sb, o_ps[:, KD : KD + 1])
                nc.vector.tensor_scalar_mul(out_all[:, qb, :], o_ps[:, 0:KD], r_sb)
                if qb % 4 == 3:
                    nc.sync.dma_start(
                        out=out_r[:, qb - 3 : qb + 1, :],
                        in_=out_all[:, qb - 3 : qb + 1, :],
                    )

            def attn_phase(kb_lo, kb_hi):
                # keep the PE two score-blocks ahead of the exp/mask chain
                pending = list(range(kb_lo, kb_hi))
                for kb in pending[: 2]:
                    score_block(kb)
                for i, kb in enumerate(pending):
                    if i + 2 < len(pending):
                        score_block(pending[i + 2])
                    attend(kb)

            # ---- phase schedule
            proj_half(0)
            for t in range(NB // 2):
                transpose_v(t)
            attn_phase(0, 7)          # attend query blocks 0..6
            proj_half(1)
            for t in range(NB // 2, NB):
                transpose_v(t)
            attn_phase(7, NB)         # attend query blocks 7..15

    nc.finalize()
    return nc


def _prep_core_inputs(inputs):
    g = lambda k: np.asarray(inputs[k], dtype=np.float32)
    x = g("x")
    scale = 1.0 / np.sqrt(np.float32(KD))
    temp = float(np.asarray(inputs["temperature"]).reshape(-1)[0])
    alpha = scale * temp  # folded (softmax temp) * (score scale)

    wq = np.concatenate([g("Wqr"), g("Wqi")], axis=1) * (scale * alpha)
    pq = np.concatenate(
        [
            g("pos_qr") * alpha + g("bqr") * (scale * alpha),
            g("pos_qi") * alpha + g("bqi") * (scale * alpha),
        ],
        axis=1,
    ).T  # [128, S]
    wk = np.concatenate([g("Wkr"), -g("Wki")], axis=1)
    pk = np.concatenate(
        [g("pos_kr") + g("bkr"), -(g("pos_ki") + g("bki"))], axis=1
    ).T
    wv = g("Wv")
    bv = g("bv").reshape(KD, 1)

    shared = {
        "wq": np.ascontiguousarray(wq, dtype=np.float32),
        "wk": np.ascontiguousarray(wk, dtype=np.float32),
        "wv": np.ascontiguousarray(wv, dtype=np.float32),
        "pq": np.ascontiguousarray(pq, dtype=np.float32),
        "pk": np.ascontiguousarray(pk, dtype=np.float32),
        "bv": np.ascontiguousarray(bv, dtype=np.float32),
    }
    in_maps = []
    for b in range(NCORES):
        m = dict(shared)
        m["xT"] = np.ascontiguousarray(x[b].T, dtype=np.float32)
        in_maps.append(m)
    return in_maps


def kernel(**inputs):
    from concourse.bass_utils import run_bass_kernel_spmd

    nc = _CACHE.get("nc")
    if nc is None:
        nc = _CACHE["nc"] = _build_nc()
    in_maps = _prep_core_inputs(inputs)
    res = run_bass_kernel_spmd(
        nc, in_maps, core_ids=list(range(NCORES)), **TRACE_KWARGS
    )
    _CACHE["last_result"] = res
    return np.stack([res.results[b]["out"] for b in range(NCORES)], axis=0)
